# revision 5
# baseline (speedup 1.0000x reference)
"""Trainium2 Bass kernel for the Centroid (segment_reduce) problem.

new_centroid = 0.3 * (segment_sum(embed, y) / counts) + 0.7 * centroid
  embed [32768, 1024] f32, y [32768] int64 (0..999), centroid [1000, 1024] f32

Strategy (8 NeuronCores, CLASS-sharded via host-side sort — no collective):
  - host sorts the batch by label; core i gets ALL rows with label in
    [125*i, 125*(i+1)) (125 classes per core). Row counts are ~4096 +- 64,
    padded to a common multiple of 256 (count flag 0 on pad rows) so one
    SPMD program serves all cores. Cores own disjoint classes => zero
    cross-core communication; host unshard is a pure concat.
  - each core's one-hot spans 125 classes -> a single 128-class M-tile:
    KP DoubleRow fp8 matmuls accumulate sums in ONE PSUM region
    [128, 1025] (cols 0..1023 dims, col 1024 count). The embed is
    pre-scaled by 0.3 and the centroid by 0.7 on the host, so finalize is
    out = psum * (1/count) + cent, one fused scalar_tensor_tensor per
    512-col chunk.
  - embed DMA is laid out partition-contiguous per tile-group on the host
    and fetched in 6 big DMAs (128 descriptors x 4-6 KB each) alternating
    between the two HWDGE queues; iota/labels/centroid ride in one packed
    misc tensor on the gpsimd queue. The full one-hot [128, KT, 128] is
    built in a single DVE is_equal with broadcast access patterns.
"""

import numpy as np

import concourse.bacc as bacc
import concourse.mybir as mybir
import concourse.tile as tile
from concourse.bass_utils import run_bass_kernel_spmd

N_CORES = 8
C = 1000  # real classes
CPC = C // N_CORES  # 125 classes owned per core
D = 1024  # embed dim
B = 32768  # total batch
P = 128
W_IN = D + 1  # 1024 dims + count column (col 1024)
W_SB = 1040  # row stride, mult of 16 (DoubleRow step constraint)
FACTOR = 0.3

_F32 = mybir.dt.float32
_FP8 = mybir.dt.float8e4

_CACHE: dict = {}


def _group_sizes(kp: int) -> list[int]:
    """Pair-tile DMA groups: small first group so the PE starts sooner."""
    if kp <= 2:
        return [kp]
    gs = [2]
    rem = kp - 2
    while rem > 0:
        g = min(3, rem)
        gs.append(g)
        rem -= g
    return gs


def _build(kp: int):
    """kp = number of 256-row DoubleRow pair-tiles per core."""
    kt = 2 * kp
    w_misc = P + kt + D  # iota | labels | 0.7*centroid
    nc = bacc.Bacc(
        "TRN2", target_bir_lowering=False, debug=False, num_devices=N_CORES
    )
    embA = nc.dram_tensor("embA", [P, kt * W_SB], _FP8, kind="ExternalInput").ap()
    misc = nc.dram_tensor("misc", [P, w_misc], _F32, kind="ExternalInput").ap()
    out = nc.dram_tensor("out", [CPC, D], _F32, kind="ExternalOutput").ap()

    gs = _group_sizes(kp)

    with tile.TileContext(nc) as tc:
        with (
            tc.tile_pool(name="sb", bufs=1) as sb_pool,
            tc.tile_pool(name="psum", bufs=1, space="PSUM") as psum_pool,
        ):
            misc_sb = sb_pool.tile([P, w_misc], _F32, name="misc", tag="misc")
            nc.gpsimd.dma_start(out=misc_sb[:], in_=misc[:])
            iota = misc_sb[:, 0:P]
            y_all = misc_sb[:, P : P + kt]
            cent07 = misc_sb[:, P + kt :]

            # full one-hot in one DVE op via broadcast APs:
            # oh_g[p, k, c] = (y[k*128+p] == c)
            oh_g = sb_pool.tile([P, kt, P], _FP8, name="ohg", tag="ohg")
            nc.vector.tensor_tensor(
                out=oh_g[:],
                in0=iota.unsqueeze(1).broadcast_to([P, kt, P]),
                in1=y_all.unsqueeze(2).broadcast_to([P, kt, P]),
                op=mybir.AluOpType.is_equal,
            )

            grp_tiles = []
            base = 0
            for g, sz in enumerate(gs):
                t = sb_pool.tile(
                    [P, 2 * sz, W_SB], _FP8, name=f"emb{g}", tag=f"emb{g}"
                )
                dma_eng = nc.sync if g % 2 == 0 else nc.scalar
                off = base * 2 * W_SB
                dma_eng.dma_start(
                    out=t[:], in_=embA[:, off : off + 2 * sz * W_SB]
                )
                grp_tiles.append((t, base, sz))
                base += sz

            ps0 = psum_pool.tile([P, 512], _F32, name="ps0", tag="ps0")
            ps1 = psum_pool.tile([P, 512], _F32, name="ps1", tag="ps1")
            ps2 = psum_pool.tile([P, 1], _F32, name="ps2", tag="ps2")
            chunks = [(ps0, 0, 512), (ps1, 512, 512), (ps2, 1024, 1)]

            for t, base, sz in grp_tiles:
                for l in range(sz):
                    j = base + l
                    for ps, off, n in chunks:
                        nc.tensor.matmul(
                            ps[:],
                            lhsT=oh_g[:, 2 * j : 2 * j + 2, :],
                            rhs=t[:, 2 * l : 2 * l + 2, off : off + n],
                            start=(j == 0),
                            stop=(j == kp - 1),
                            perf_mode=mybir.MatmulPerfMode.DoubleRow,
                        )

            # out = sums * (1/count) + 0.7*centroid   (0.3 folded into embed)
            recip = sb_pool.tile([P, 1], _F32, name="recip", tag="recip")
            nc.vector.reciprocal(recip[:], ps2[:])
            for q, (ps, off, n) in enumerate(chunks[:2]):
                out_sb = sb_pool.tile([P, 512], _F32, name=f"o{q}", tag=f"o{q}")
                nc.vector.scalar_tensor_tensor(
                    out_sb[0:CPC, :],
                    ps[0:CPC, :],
                    recip[0:CPC, 0:1],
                    cent07[0:CPC, off : off + n],
                    mybir.AluOpType.mult,
                    mybir.AluOpType.add,
                )
                dma_eng = nc.sync if q == 0 else nc.scalar
                dma_eng.dma_start(
                    out=out[:, off : off + n], in_=out_sb[0:CPC, :]
                )

    nc.compile()
    return nc


def get_nc(kp: int):
    if kp not in _CACHE:
        _CACHE[kp] = _build(kp)
    return _CACHE[kp]


def prepare(embed: np.ndarray, y: np.ndarray, centroid: np.ndarray):
    """Sort batch by label, shard class-aligned, pad, quantize, lay out
    partition-contiguous. Returns (nc, in_maps)."""
    y = np.asarray(y).astype(np.int64).ravel()
    order = np.argsort(y, kind="stable")
    ys = y[order]
    bounds = np.searchsorted(ys, np.arange(0, C + 1, CPC))
    n_max = int(np.diff(bounds).max())
    kp = max((n_max + 255) // 256, 1)
    rows = kp * 256
    kt = 2 * kp
    w_misc = P + kt + D

    fp8 = mybir.dt.np(_FP8)
    embf = np.asarray(embed, dtype=np.float32) * FACTOR
    cent07 = np.asarray(centroid, dtype=np.float32) * (1.0 - FACTOR)
    one8 = np.float32(1.0).astype(fp8)

    iota_np = np.tile(np.arange(P, dtype=np.float32), (P, 1))
    in_maps = []
    for i in range(N_CORES):
        lo, hi = int(bounds[i]), int(bounds[i + 1])
        n = hi - lo
        idx = order[lo:hi]
        e = np.zeros((rows, W_SB), dtype=fp8)
        e[:n, :D] = embf[idx].astype(fp8)
        e[:n, D] = one8
        # partition-contiguous: embA[p, k*W_SB : (k+1)*W_SB] = row k*128+p
        embA = np.ascontiguousarray(
            e.reshape(kt, P, W_SB).transpose(1, 0, 2).reshape(P, kt * W_SB)
        )
        y_rel = np.zeros(rows, dtype=np.float32)
        y_rel[:n] = (ys[lo:hi] - CPC * i).astype(np.float32)
        misc = np.zeros((P, w_misc), dtype=np.float32)
        misc[:, 0:P] = iota_np
        misc[:, P : P + kt] = y_rel.reshape(kt, P).T
        misc[0:CPC, P + kt :] = cent07[CPC * i : CPC * (i + 1)]
        in_maps.append({"embA": embA, "misc": misc})
    return get_nc(kp), in_maps


def assemble(res) -> np.ndarray:
    full = np.concatenate(
        [res.results[i]["out"] for i in range(N_CORES)], axis=0
    )
    return np.ascontiguousarray(full).astype(np.float32)


def kernel(embed: np.ndarray, y: np.ndarray, centroid: np.ndarray) -> np.ndarray:
    nc, in_maps = prepare(embed, y, centroid)
    res = run_bass_kernel_spmd(nc, in_maps, core_ids=list(range(N_CORES)))
    return assemble(res)


# revision 10
# speedup vs baseline: 1.1842x; 1.1842x over previous
"""Trainium2 Bass kernel for the Centroid (segment_reduce) problem.

new_centroid = 0.3 * (segment_sum(embed, y) / counts) + 0.7 * centroid
  embed [32768, 1024] f32, y [32768] int64 (0..999), centroid [1000, 1024] f32

Strategy (8 NeuronCores, CLASS-sharded via host-side sort — no collective):
  - host sorts the batch by label; core i gets ALL rows with label in
    [125*i, 125*(i+1)) (125 classes per core). Row counts are ~4096 +- 64,
    padded to a common multiple of 256 (count flag 0 on pad rows) so one
    SPMD program serves all cores. Cores own disjoint classes => zero
    cross-core communication; host unshard is a pure concat.
  - each core's one-hot spans 125 classes -> a single 128-class M-tile:
    KP DoubleRow fp8 matmuls accumulate sums in ONE PSUM region
    [128, 1025] (cols 0..1023 dims, col 1024 count). The embed is
    pre-scaled by 0.3 and the centroid by 0.7 on the host, so finalize is
    out = psum * (1/count) + cent, one fused scalar_tensor_tensor per
    512-col chunk.
  - embed DMA is laid out partition-contiguous per tile-group on the host
    and fetched in 6 big DMAs (128 descriptors x 4-6 KB each) alternating
    between the two HWDGE queues; iota/labels/centroid ride in one packed
    misc tensor on the gpsimd queue. The full one-hot [128, KT, 128] is
    built in a single DVE is_equal with broadcast access patterns.
"""

import numpy as np

import concourse.bacc as bacc
import concourse.mybir as mybir
import concourse.tile as tile
from concourse.bass_utils import run_bass_kernel_spmd

N_CORES = 8
C = 1000  # real classes
CPC = C // N_CORES  # 125 classes owned per core
D = 1024  # embed dim
B = 32768  # total batch
P = 128
W_IN = D + 1  # 1024 dims + count column (col 1024)
W_SB = 1040  # row stride, mult of 16 (DoubleRow step constraint)
FACTOR = 0.3

_F32 = mybir.dt.float32
_FP8 = mybir.dt.float8e4

_CACHE: dict = {}


def _group_sizes(kp: int) -> list[int]:
    """Pair-tile DMA groups: small first group so the PE starts sooner."""
    if kp <= 2:
        return [kp]
    gs = [2]
    rem = kp - 2
    while rem > 0:
        g = min(3, rem)
        gs.append(g)
        rem -= g
    return gs


def _build(kp: int):
    """kp = number of 256-row DoubleRow pair-tiles per core."""
    kt = 2 * kp
    nc = bacc.Bacc(
        "TRN2", target_bir_lowering=False, debug=False, num_devices=N_CORES
    )
    embA = nc.dram_tensor("embA", [P, kt * W_SB], _FP8, kind="ExternalInput").ap()
    lab = nc.dram_tensor("lab", [P, P + kt], _F32, kind="ExternalInput").ap()
    cent = nc.dram_tensor("cent", [CPC, D], _F32, kind="ExternalInput").ap()
    out = nc.dram_tensor("out", [CPC, D], _F32, kind="ExternalOutput").ap()

    gs = _group_sizes(kp)

    with tile.TileContext(nc) as tc:
        with (
            tc.tile_pool(name="sb", bufs=1) as sb_pool,
            tc.tile_pool(name="psum", bufs=1, space="PSUM") as psum_pool,
        ):
            # tiny labels tensor FIRST on a HWDGE queue so the one-hot build
            # isn't stuck behind the 4.5 MB embed stream
            lab_sb = sb_pool.tile([P, P + kt], _F32, name="lab", tag="lab")
            nc.sync.dma_start(out=lab_sb[:], in_=lab[:])
            iota = lab_sb[:, 0:P]
            y_all = lab_sb[:, P : P + kt]

            # 0.7*centroid (host-prescaled), needed only at finalize
            cent07 = sb_pool.tile([P, D], _F32, name="cent", tag="cent")
            nc.gpsimd.dma_start(out=cent07[0:CPC, :], in_=cent[:])

            grp_tiles = []
            base = 0
            for g, sz in enumerate(gs):
                t = sb_pool.tile(
                    [P, 2 * sz, W_SB], _FP8, name=f"emb{g}", tag=f"emb{g}"
                )
                dma_eng = nc.scalar if g % 2 == 0 else nc.sync
                off = base * 2 * W_SB
                dma_eng.dma_start(
                    out=t[:], in_=embA[:, off : off + 2 * sz * W_SB]
                )
                grp_tiles.append((t, base, sz))
                base += sz

            # per-pair one-hot builds: oh_g[p, k, c] = (y[k*128+p] == c);
            # fine-grained so matmuls pipeline with DMA arrivals
            oh_g = sb_pool.tile([P, kt, P], _FP8, name="ohg", tag="ohg")
            for j in range(kp):
                nc.vector.tensor_tensor(
                    out=oh_g[:, 2 * j : 2 * j + 2, :],
                    in0=iota.unsqueeze(1).broadcast_to([P, 2, P]),
                    in1=y_all[:, 2 * j : 2 * j + 2]
                    .unsqueeze(2)
                    .broadcast_to([P, 2, P]),
                    op=mybir.AluOpType.is_equal,
                )

            ps0 = psum_pool.tile([P, 512], _F32, name="ps0", tag="ps0")
            ps1 = psum_pool.tile([P, 512], _F32, name="ps1", tag="ps1")
            ps2 = psum_pool.tile([P, 1], _F32, name="ps2", tag="ps2")
            chunks = [(ps0, 0, 512), (ps1, 512, 512), (ps2, 1024, 1)]

            for t, base, sz in grp_tiles:
                for l in range(sz):
                    j = base + l
                    for ps, off, n in chunks:
                        nc.tensor.matmul(
                            ps[:],
                            lhsT=oh_g[:, 2 * j : 2 * j + 2, :],
                            rhs=t[:, 2 * l : 2 * l + 2, off : off + n],
                            start=(j == 0),
                            stop=(j == kp - 1),
                            perf_mode=mybir.MatmulPerfMode.DoubleRow,
                        )

            # out = sums * (1/count) + 0.7*centroid   (0.3 folded into embed)
            recip = sb_pool.tile([P, 1], _F32, name="recip", tag="recip")
            nc.vector.reciprocal(recip[:], ps2[:])
            for q, (ps, off, n) in enumerate(chunks[:2]):
                out_sb = sb_pool.tile([P, 512], _F32, name=f"o{q}", tag=f"o{q}")
                nc.vector.scalar_tensor_tensor(
                    out_sb[0:CPC, :],
                    ps[0:CPC, :],
                    recip[0:CPC, 0:1],
                    cent07[0:CPC, off : off + n],
                    mybir.AluOpType.mult,
                    mybir.AluOpType.add,
                )
                dma_eng = nc.sync if q == 0 else nc.scalar
                dma_eng.dma_start(
                    out=out[:, off : off + n], in_=out_sb[0:CPC, :]
                )

    nc.compile()
    return nc


def get_nc(kp: int):
    if kp not in _CACHE:
        _CACHE[kp] = _build(kp)
    return _CACHE[kp]


def prepare(embed: np.ndarray, y: np.ndarray, centroid: np.ndarray):
    """Sort batch by label, shard class-aligned, pad, quantize, lay out
    partition-contiguous. Returns (nc, in_maps)."""
    y = np.asarray(y).astype(np.int64).ravel()
    order = np.argsort(y, kind="stable")
    ys = y[order]
    bounds = np.searchsorted(ys, np.arange(0, C + 1, CPC))
    n_max = int(np.diff(bounds).max())
    kp = max((n_max + 255) // 256, 1)
    rows = kp * 256
    kt = 2 * kp

    fp8 = mybir.dt.np(_FP8)
    embf = np.asarray(embed, dtype=np.float32) * FACTOR
    cent07 = np.asarray(centroid, dtype=np.float32) * (1.0 - FACTOR)
    one8 = np.float32(1.0).astype(fp8)

    iota_np = np.tile(np.arange(P, dtype=np.float32), (P, 1))
    in_maps = []
    for i in range(N_CORES):
        lo, hi = int(bounds[i]), int(bounds[i + 1])
        n = hi - lo
        idx = order[lo:hi]
        e = np.zeros((rows, W_SB), dtype=fp8)
        e[:n, :D] = embf[idx].astype(fp8)
        e[:n, D] = one8
        # partition-contiguous: embA[p, k*W_SB : (k+1)*W_SB] = row k*128+p
        embA = np.ascontiguousarray(
            e.reshape(kt, P, W_SB).transpose(1, 0, 2).reshape(P, kt * W_SB)
        )
        y_rel = np.zeros(rows, dtype=np.float32)
        y_rel[:n] = (ys[lo:hi] - CPC * i).astype(np.float32)
        lab = np.empty((P, P + kt), dtype=np.float32)
        lab[:, 0:P] = iota_np
        lab[:, P : P + kt] = y_rel.reshape(kt, P).T
        in_maps.append(
            {
                "embA": embA,
                "lab": lab,
                "cent": np.ascontiguousarray(cent07[CPC * i : CPC * (i + 1)]),
            }
        )
    return get_nc(kp), in_maps


def assemble(res) -> np.ndarray:
    full = np.concatenate(
        [res.results[i]["out"] for i in range(N_CORES)], axis=0
    )
    return np.ascontiguousarray(full).astype(np.float32)


def kernel(embed: np.ndarray, y: np.ndarray, centroid: np.ndarray) -> np.ndarray:
    nc, in_maps = prepare(embed, y, centroid)
    res = run_bass_kernel_spmd(nc, in_maps, core_ids=list(range(N_CORES)))
    return assemble(res)
